# revision 10
# baseline (speedup 1.0000x reference)
"""Trainium2 Bass kernel for nn_Cross_attention (dual-stream cross attention).

Shape summary (per stream): x (B=4, C=128, 64, 64) -> GroupNorm(16 groups) ->
QKV 1x1 conv (4 heads, d=32) -> cross attention over HW=4096 positions
(q from the *other* stream) -> out 1x1 conv + bias -> + normalized input.

Sharding: 8 independent (batch, output-side) units -> one per NeuronCore.
Core i < 4 computes out_A[i] (k/v/residual from stream A, q from stream B);
core i >= 4 computes out_B[i-4].

Per-core dataflow (all matmuls float32r, full 128x128 PE rate at N>=256):
  - GroupNorm stats via bn_stats/bn_aggr per channel, group-combined and
    broadcast back with tiny mask matmuls; affine folded into (x*s1 + s2).
  - q, k projected into (128=[head,d], 4096) layout; v projected transposed
    into (4096, 128=[head,d]) layout (lhsT for the PV matmul).
  - Scores computed transposed: S^T(k,q) = k^T q via 4 row-tiled (K=32)
    matmuls, one PE pass for all 4 heads.
  - exp() with no max-subtraction (scores are provably tiny: |s| <~ 0.3);
    split between ScalarE table-exp and a single-pass custom-DVE quartic
    polynomial so both engines stream the 67M-element attention matrix.
  - O = (exp S^T)^T-contracted with v via col-tiled matmuls; softmax
    denominators via a parallel ones-matmul into a second PSUM bank
    (replicated across each head's 32 partitions); normalize with
    reciprocal_approx_fast at the (128, 512) output tile level.
  - out = W_out @ (O/den) + b + xn, DMA'd back per 512-column chunk.
"""

import math
import os
import sys

sys.path.insert(0, "/opt/trn_rl_repo")

import numpy as np

C = 128
N_HEAD = 4
D = 32
GROUPS = 16
EPS = 1e-5
B = 4
HW = 4096
NQ = 512            # q-chunk (free dim of score matmuls / PSUM bank)
NJ = HW // NQ       # 8 q-chunks
KC = 128            # k-chunk (partition dim of exp'd score tiles)
NKC = HW // KC      # 32 k-chunks
N_CORES = 8

# exp engine split: of every 8 (kc, head-pair) chunks, this many go to ScalarE
# (table exp), the rest to the VectorE quartic-poly custom op.
ACT_OF_8 = 5

_EXP4 = None
_COMPILED = {}


def _register_exp4():
    """Register a quartic-Taylor exp as a custom DVE op (single uop pass).

    exp(x) ~= 1 + x(1 + x(1/2 + x(1/6 + x/24))); scores satisfy |x| <~ 0.3
    so max rel err ~4e-5 (4e-4 at |x|=0.5) - far below fp32r matmul noise.
    """
    global _EXP4
    if _EXP4 is not None:
        return _EXP4
    from concourse import dve_ops
    from concourse.dve_spec import C0, C1, C2, One, Spec, Src0, lower
    from concourse.dve_uop import DveOpSpec

    for op in dve_ops.OPS:
        if op.name == "EXP4_ANT":
            _EXP4 = op
            return op

    body = ((((Src0 * C0 + C1) * Src0 + C2) * Src0 + One) * Src0 + One)
    spec = Spec(
        body=body,
        reference=lambda in0, in1, s0, s1, imm2: (
            (((in0.astype(np.float32) * s0 + s1) * in0 + imm2) * in0 + 1.0)
            * in0 + 1.0
        ),
    )
    row = dve_ops._CUSTOM_DVE_ROW_BASE + len(dve_ops.OPS)
    assert row < 0x20
    dve_ops._SUB_OPCODE_FOR_NAME["EXP4_ANT"] = row
    shas = {}
    for ver in ("v3", "v4"):
        uops = lower(spec, ver=ver)
        shas[ver] = DveOpSpec(
            name="EXP4_ANT", opcode=row, uops=uops, rd1_en=False
        ).sha(ver)
    op = dve_ops.DveOp("EXP4_ANT", spec, subdim=False, uops_sha=shas)
    dve_ops.OPS.append(op)
    dve_ops.CUSTOM_DVE_SPECS["EXP4_ANT"] = spec
    _EXP4 = op
    return op


def build_nc(repeat: int = 1):
    """Build + compile the SPMD single-core program (same for all 8 cores)."""
    import concourse.bacc as bacc
    import concourse.tile as tile
    from concourse import mybir

    exp4 = _register_exp4()
    f32 = mybir.dt.float32
    f32r = mybir.dt.float32r
    bf16 = mybir.dt.bfloat16
    AF = mybir.ActivationFunctionType
    OP = mybir.AluOpType

    def r(ap):
        return ap

    nc = bacc.Bacc("TRN2", target_bir_lowering=False, debug=False,
                   num_devices=N_CORES)

    x_kv_d = nc.dram_tensor("x_kv", [C, HW], f32r, kind="ExternalInput")
    x_q_d = nc.dram_tensor("x_q", [C, HW], f32r, kind="ExternalInput")
    wq_d = nc.dram_tensor("wq_t", [C, C], f32r, kind="ExternalInput")
    wk_d = nc.dram_tensor("wk_t", [C, C], f32r, kind="ExternalInput")
    wv_d = nc.dram_tensor("wv_t", [C, C], f32r, kind="ExternalInput")
    wo_d = nc.dram_tensor("wo_t", [C, C], f32r, kind="ExternalInput")
    bo_d = nc.dram_tensor("b_out", [C, 1], f32, kind="ExternalInput")
    gnkv_d = nc.dram_tensor("gn_kv", [C, 2], f32, kind="ExternalInput")
    gnq_d = nc.dram_tensor("gn_q", [C, 2], f32, kind="ExternalInput")
    gm_d = nc.dram_tensor("gmask", [C, GROUPS], f32, kind="ExternalInput")
    gmt_d = nc.dram_tensor("gmask_t", [GROUPS, C], f32, kind="ExternalInput")
    ones_d = nc.dram_tensor("ones32", [C, D], bf16, kind="ExternalInput")
    out_d = nc.dram_tensor("out", [C, HW], f32, kind="ExternalOutput")

    with tile.TileContext(nc) as tc:
        with (
            tc.tile_pool(name="const", bufs=1) as cpool,
            tc.tile_pool(name="big", bufs=1) as big,
            tc.tile_pool(name="small", bufs=2) as small,
            tc.tile_pool(name="exps", bufs=6) as epool,
            tc.tile_pool(name="epi", bufs=2) as epi,
            tc.tile_pool(name="pseq", bufs=2, space="PSUM") as pseq,
            tc.tile_pool(name="pscore", bufs=2, space="PSUM") as pscore,
            tc.tile_pool(name="pacc", bufs=1, space="PSUM") as pacc,
        ):
            # ---- constants ----
            wq_t = cpool.tile([C, C], f32r, tag="wq")
            wk_t = cpool.tile([C, C], f32r, tag="wk")
            wv_t = cpool.tile([C, C], f32r, tag="wv")
            wo_t = cpool.tile([C, C], f32r, tag="wo")
            b_out = cpool.tile([C, 1], f32, tag="bo")
            gn_kv = cpool.tile([C, 2], f32, tag="gnkv")
            gn_q = cpool.tile([C, 2], f32, tag="gnq")
            gmask = cpool.tile([C, GROUPS], f32, tag="gm")
            gmask_t = cpool.tile([GROUPS, C], f32, tag="gmt")
            eps_t = cpool.tile([C, 1], f32, tag="eps")
            ones32 = cpool.tile([C, D], bf16, tag="ones")
            for t, d in ((wq_t, wq_d), (wk_t, wk_d), (wv_t, wv_d),
                         (wo_t, wo_d), (b_out, bo_d), (gn_kv, gnkv_d),
                         (gn_q, gnq_d), (gmask, gm_d), (gmask_t, gmt_d),
                         (ones32, ones_d)):
                nc.sync.dma_start(out=t[:], in_=d[:])
            nc.vector.memset(eps_t[:], EPS)

            # ---- persistent activations ----
            x_kv = big.tile([C, HW], f32r, tag="x_kv")
            x_q = big.tile([C, HW], f32r, tag="x_q")
            qsb = big.tile([C, HW], bf16, tag="qsb")
            ksb = big.tile([C, HW], bf16, tag="ksb")
            vts = big.tile([C, NKC, C], bf16, tag="vts")  # v^T (hw, [h,d])
            nc.sync.dma_start(out=x_kv[:], in_=x_kv_d[:])
            nc.sync.dma_start(out=x_q[:], in_=x_q_d[:])

            for _rep in range(repeat):
                # ---- GroupNorm (in place) on x_kv and x_q ----
                for x_sb, gn in ((x_kv, gn_kv), (x_q, gn_q)):
                    xs = x_sb[:].rearrange("p (s f) -> p s f", f=512)
                    stats = small.tile([C, 8, 6], f32, tag="gnstats")
                    for s in range(8):
                        nc.vector.bn_stats(out=stats[:, s, :], in_=xs[:, s, :])
                    mv = small.tile([C, 2], f32, tag="gnmv")
                    nc.vector.bn_aggr(out=mv[:], in_=stats[:])
                    # per-channel [mean, E[x^2]]
                    st2 = small.tile([C, 2], f32, tag="gnst2")
                    nc.vector.tensor_copy(st2[:, 0:1], mv[:, 0:1])
                    nc.vector.tensor_tensor(st2[:, 1:2], mv[:, 0:1],
                                            mv[:, 0:1], OP.mult)
                    nc.vector.tensor_tensor(st2[:, 1:2], st2[:, 1:2],
                                            mv[:, 1:2], OP.add)
                    # group combine: (16, 2) = gmask.T @ st2
                    pg = pseq.tile([GROUPS, 2], f32, tag="seq")
                    nc.tensor.matmul(pg[:], lhsT=gmask[:], rhs=st2[:],
                                     start=True, stop=True)
                    gstat = small.tile([GROUPS, 2], f32, tag="gstat")
                    nc.vector.tensor_copy(gstat[:], pg[:])
                    # var_g = E[x^2]_g - mean_g^2 ; rstd = 1/sqrt(var+eps)
                    vtmp = small.tile([GROUPS, 1], f32, tag="gvtmp")
                    nc.vector.tensor_tensor(vtmp[:], gstat[:, 0:1],
                                            gstat[:, 0:1], OP.mult)
                    nc.vector.tensor_tensor(gstat[:, 1:2], gstat[:, 1:2],
                                            vtmp[:], OP.subtract)
                    nc.scalar.activation(out=gstat[:, 1:2], in_=gstat[:, 1:2],
                                         func=AF.Sqrt, bias=eps_t[:GROUPS])
                    nc.vector.reciprocal(out=gstat[:, 1:2], in_=gstat[:, 1:2])
                    # broadcast to channels: (128, 2) = gmask_t.T @ gstat
                    pcb = pseq.tile([C, 2], f32, tag="seq")
                    nc.tensor.matmul(pcb[:], lhsT=gmask_t[:], rhs=gstat[:],
                                     start=True, stop=True)
                    # fold affine: s1 = rstd*w ; s2 = b - mean*s1
                    s1 = small.tile([C, 1], f32, tag="gns1")
                    s2 = small.tile([C, 1], f32, tag="gns2")
                    nc.vector.tensor_tensor(s1[:], pcb[:, 1:2], gn[:, 0:1],
                                            OP.mult)
                    nc.vector.tensor_tensor(s2[:], pcb[:, 0:1], s1[:], OP.mult)
                    nc.vector.tensor_tensor(s2[:], gn[:, 1:2], s2[:],
                                            OP.subtract)
                    nc.vector.tensor_scalar(out=x_sb[:], in0=x_sb[:],
                                            scalar1=s1[:], scalar2=s2[:],
                                            op0=OP.mult, op1=OP.add)

                # ---- projections ----
                for j in range(NJ):
                    sl = slice(j * NQ, (j + 1) * NQ)
                    pq = pseq.tile([C, NQ], f32, tag="seq")
                    nc.tensor.matmul(pq[:], lhsT=r(wq_t[:]), rhs=r(x_q[:, sl]),
                                     start=True, stop=True)
                    nc.scalar.copy(out=qsb[:, sl], in_=pq[:])
                    pk = pseq.tile([C, NQ], f32, tag="seq")
                    nc.tensor.matmul(pk[:], lhsT=r(wk_t[:]), rhs=r(x_kv[:, sl]),
                                     start=True, stop=True)
                    nc.vector.tensor_copy(ksb[:, sl], pk[:])
                for j in range(NJ):  # v^T in 4 kc-chunks per PSUM bank
                    pv = pseq.tile([C, 4, C], f32, tag="seq")
                    for t in range(4):
                        kc = j * 4 + t
                        nc.tensor.matmul(
                            pv[:, t, :],
                            lhsT=r(x_kv[:, kc * KC:(kc + 1) * KC]),
                            rhs=r(wv_t[:]), start=True, stop=True)
                    nc.vector.tensor_copy(vts[:, j * 4:(j + 1) * 4, :], pv[:])

                # ---- attention ----
                expctr = 0
                for j in range(NJ):
                    qs = slice(j * NQ, (j + 1) * NQ)
                    po = pacc.tile([C, NQ], f32, tag="po")
                    pd = pacc.tile([C, NQ], f32, tag="pd")
                    for kc in range(NKC):
                        ks = slice(kc * KC, (kc + 1) * KC)
                        first, last = kc == 0, kc == NKC - 1
                        for pair in range(2):
                            ps = pscore.tile([C, 2, NQ], f32, tag="ps")
                            for hh in range(2):
                                h = pair * 2 + hh
                                hs = slice(h * D, (h + 1) * D)
                                nc.tensor.matmul(
                                    ps[:, hh, :],
                                    lhsT=r(ksb[hs, ks]),
                                    rhs=r(qsb[hs, qs]),
                                    start=True, stop=True,
                                    tile_position=(h * D, 0))
                            ex = epool.tile([C, 2, NQ], bf16, tag="ex")
                            if (expctr * ACT_OF_8) % 8 < ACT_OF_8:
                                nc.scalar.activation(out=ex[:], in_=ps[:],
                                                     func=AF.Exp)
                            else:
                                nc.vector._custom_dve(
                                    exp4, out=ex[:], in0=ps[:],
                                    s0=1.0 / 24.0, s1=1.0 / 6.0, imm2=0.5)
                            expctr += 1
                            for hh in range(2):
                                h = pair * 2 + hh
                                hs = slice(h * D, (h + 1) * D)
                                nc.tensor.matmul(
                                    po[hs, :],
                                    lhsT=r(vts[:, kc, hs]),
                                    rhs=r(ex[:, hh, :]),
                                    start=first, stop=last,
                                    skip_group_check=True,
                                    tile_position=(0, h * D))
                                nc.tensor.matmul(
                                    pd[hs, :],
                                    lhsT=r(ones32[:]),
                                    rhs=r(ex[:, hh, :]),
                                    start=first, stop=last,
                                    skip_group_check=True,
                                    tile_position=(0, h * D))
                    # ---- epilogue for this q-chunk ----
                    recip = epi.tile([C, NQ], f32, tag="recip")
                    nc.vector.reciprocal_approx_fast(out=recip[:], in_=pd[:])
                    onorm = epi.tile([C, NQ], f32r, tag="onorm")
                    nc.vector.tensor_tensor(onorm[:], po[:], recip[:], OP.mult)
                    pout = pseq.tile([C, NQ], f32, tag="seq")
                    nc.tensor.matmul(pout[:], lhsT=r(wo_t[:]), rhs=r(onorm[:]),
                                     start=True, stop=True)
                    ot = epi.tile([C, NQ], f32, tag="ot")
                    nc.vector.tensor_scalar_add(ot[:], pout[:], b_out[:])
                    nc.vector.tensor_tensor(ot[:], ot[:], x_kv[:, qs], OP.add)
                    nc.sync.dma_start(out=out_d[:, qs], in_=ot[:])

    nc.compile()
    return nc


import ml_dtypes
_bf16_np = ml_dtypes.bfloat16


def _prep_in_maps(x_A, x_B, gnA_w, gnA_b, gnB_w, gnB_b, W_qkv_A, W_qkv_B,
                  W_out_A, b_out_A, W_out_B, b_out_B):
    """Host-side shard: one in_map per core; constants pre-transposed."""
    f = np.float32
    # bn stats are already means over HW; the group combine averages the 8
    # member channels of each group, so the combine weight is 1/8.
    gmask = np.zeros((C, GROUPS), f)
    gmask_t = np.zeros((GROUPS, C), f)
    for c in range(C):
        gmask[c, c // 8] = 1.0 / 8.0
        gmask_t[c // 8, c] = 1.0
    scale = 1.0 / math.sqrt(C)

    def side_maps(x_self, x_other, Wqkv_self, Wqkv_other, gn_s_w, gn_s_b,
                  gn_o_w, gn_o_b, W_out, b_out):
        qkv_s = np.asarray(Wqkv_self, f).reshape(N_HEAD, 3, D, C)
        qkv_o = np.asarray(Wqkv_other, f).reshape(N_HEAD, 3, D, C)
        wq_t = np.ascontiguousarray(
            (qkv_o[:, 0].reshape(C, C) * scale).T)
        wk_t = np.ascontiguousarray(qkv_s[:, 1].reshape(C, C).T)
        wv_t = np.ascontiguousarray(qkv_s[:, 2].reshape(C, C).T)
        wo_t = np.ascontiguousarray(np.asarray(W_out, f).T)
        common = {
            "wq_t": wq_t, "wk_t": wk_t, "wv_t": wv_t, "wo_t": wo_t,
            "b_out": np.asarray(b_out, f).reshape(C, 1),
            "gn_kv": np.stack([np.asarray(gn_s_w, f),
                               np.asarray(gn_s_b, f)], axis=1),
            "gn_q": np.stack([np.asarray(gn_o_w, f),
                              np.asarray(gn_o_b, f)], axis=1),
            "gmask": gmask, "gmask_t": gmask_t,
            "ones32": np.ones((C, D), _bf16_np),
        }
        return [
            dict(common,
                 x_kv=np.ascontiguousarray(
                     np.asarray(x_self[b], f).reshape(C, HW)),
                 x_q=np.ascontiguousarray(
                     np.asarray(x_other[b], f).reshape(C, HW)))
            for b in range(B)
        ]

    maps = side_maps(x_A, x_B, W_qkv_A, W_qkv_B, gnA_w, gnA_b, gnB_w, gnB_b,
                     W_out_A, b_out_A)
    maps += side_maps(x_B, x_A, W_qkv_B, W_qkv_A, gnB_w, gnB_b, gnA_w, gnA_b,
                      W_out_B, b_out_B)
    return maps


def get_compiled(repeat: int = 1):
    if repeat not in _COMPILED:
        _COMPILED[repeat] = build_nc(repeat)
    return _COMPILED[repeat]


def run_on_cores(in_maps, repeat: int = 1):
    from concourse.bass_utils import run_bass_kernel_spmd
    nc = get_compiled(repeat)
    res = run_bass_kernel_spmd(nc, in_maps, core_ids=list(range(N_CORES)))
    return res.results


def kernel(**inputs):
    in_maps = _prep_in_maps(**{k: np.asarray(v) for k, v in inputs.items()})
    results = run_on_cores(in_maps, repeat=int(os.environ.get("CA_REPEAT", "1")))
    out_A = np.stack([results[b]["out"].reshape(C, 64, 64) for b in range(B)])
    out_B = np.stack([results[B + b]["out"].reshape(C, 64, 64)
                      for b in range(B)])
    return out_A.astype(np.float32), out_B.astype(np.float32)


# revision 12
# speedup vs baseline: 2.8328x; 2.8328x over previous
"""Trainium2 Bass kernel for nn_Cross_attention (dual-stream cross attention).

Shape summary (per stream): x (B=4, C=128, 64, 64) -> GroupNorm(16 groups) ->
QKV 1x1 conv (4 heads, d=32) -> cross attention over HW=4096 positions
(q from the *other* stream) -> out 1x1 conv + bias -> + normalized input.

Sharding: 8 independent (batch, output-side) units -> one per NeuronCore.
Core i < 4 computes out_A[i] (k/v/residual from stream A, q from stream B);
core i >= 4 computes out_B[i-4].

Per-core dataflow (all matmuls float32r, full 128x128 PE rate at N>=256):
  - GroupNorm stats via bn_stats/bn_aggr per channel, group-combined and
    broadcast back with tiny mask matmuls; affine folded into (x*s1 + s2).
  - q, k projected into (128=[head,d], 4096) layout; v projected transposed
    into (4096, 128=[head,d]) layout (lhsT for the PV matmul).
  - Scores computed transposed: S^T(k,q) = k^T q via 4 row-tiled (K=32)
    matmuls, one PE pass for all 4 heads.
  - exp() with no max-subtraction (scores are provably tiny: |s| <~ 0.3);
    split between ScalarE table-exp and a single-pass custom-DVE quartic
    polynomial so both engines stream the 67M-element attention matrix.
  - O = (exp S^T)^T-contracted with v via col-tiled matmuls; softmax
    denominators via a parallel ones-matmul into a second PSUM bank
    (replicated across each head's 32 partitions); normalize with
    reciprocal_approx_fast at the (128, 512) output tile level.
  - out = W_out @ (O/den) + b + xn, DMA'd back per 512-column chunk.
"""

import math
import os
import sys

sys.path.insert(0, "/opt/trn_rl_repo")

import numpy as np

C = 128
N_HEAD = 4
D = 32
GROUPS = 16
EPS = 1e-5
B = 4
HW = 4096
NQ = 512            # q-chunk (free dim of score matmuls / PSUM bank)
NJ = HW // NQ       # 8 q-chunks
KC = 128            # k-chunk (partition dim of exp'd score tiles)
NKC = HW // KC      # 32 k-chunks
N_CORES = 8

# exp engine split: of every 8 (kc, head-pair) chunks, this many go to ScalarE
# (table exp), the rest to the VectorE quartic-poly custom op.
ACT_OF_8 = int(os.environ.get("CA_ACT_OF_8", "5"))
EPOOL_BUFS = int(os.environ.get("CA_EPOOL_BUFS", "6"))

_EXP4 = None
_COMPILED = {}


def _register_exp4():
    """Register a quartic-Taylor exp as a custom DVE op (single uop pass).

    exp(x) ~= 1 + x(1 + x(1/2 + x(1/6 + x/24))); scores satisfy |x| <~ 0.3
    so max rel err ~4e-5 (4e-4 at |x|=0.5) - far below fp32r matmul noise.
    """
    global _EXP4
    if _EXP4 is not None:
        return _EXP4
    from concourse import dve_ops
    from concourse.dve_spec import C0, C1, C2, One, Spec, Src0, lower
    from concourse.dve_uop import DveOpSpec

    for op in dve_ops.OPS:
        if op.name == "EXP4_ANT":
            _EXP4 = op
            return op

    body = ((((Src0 * C0 + C1) * Src0 + C2) * Src0 + One) * Src0 + One)
    spec = Spec(
        body=body,
        reference=lambda in0, in1, s0, s1, imm2: (
            (((in0.astype(np.float32) * s0 + s1) * in0 + imm2) * in0 + 1.0)
            * in0 + 1.0
        ),
    )
    row = dve_ops._CUSTOM_DVE_ROW_BASE + len(dve_ops.OPS)
    assert row < 0x20
    dve_ops._SUB_OPCODE_FOR_NAME["EXP4_ANT"] = row
    shas = {}
    for ver in ("v3", "v4"):
        uops = lower(spec, ver=ver)
        shas[ver] = DveOpSpec(
            name="EXP4_ANT", opcode=row, uops=uops, rd1_en=False
        ).sha(ver)
    op = dve_ops.DveOp("EXP4_ANT", spec, subdim=False, uops_sha=shas)
    dve_ops.OPS.append(op)
    dve_ops.CUSTOM_DVE_SPECS["EXP4_ANT"] = spec
    _EXP4 = op
    return op


def build_nc(repeat: int = 1):
    """Build + compile the SPMD single-core program (same for all 8 cores)."""
    import concourse.bacc as bacc
    import concourse.tile as tile
    from concourse import mybir

    exp4 = _register_exp4()
    f32 = mybir.dt.float32
    f32r = mybir.dt.float32r
    bf16 = mybir.dt.bfloat16
    AF = mybir.ActivationFunctionType
    OP = mybir.AluOpType

    def r(ap):
        return ap

    nc = bacc.Bacc("TRN2", target_bir_lowering=False, debug=False,
                   num_devices=N_CORES)

    x_kv_d = nc.dram_tensor("x_kv", [C, HW], f32r, kind="ExternalInput")
    x_q_d = nc.dram_tensor("x_q", [C, HW], f32r, kind="ExternalInput")
    wq_d = nc.dram_tensor("wq_t", [C, C], f32r, kind="ExternalInput")
    wk_d = nc.dram_tensor("wk_t", [C, C], f32r, kind="ExternalInput")
    wv_d = nc.dram_tensor("wv_t", [C, C], f32r, kind="ExternalInput")
    wo_d = nc.dram_tensor("wo_t", [C, C], f32r, kind="ExternalInput")
    bo_d = nc.dram_tensor("b_out", [C, 1], f32, kind="ExternalInput")
    gnkv_d = nc.dram_tensor("gn_kv", [C, 2], f32, kind="ExternalInput")
    gnq_d = nc.dram_tensor("gn_q", [C, 2], f32, kind="ExternalInput")
    gm_d = nc.dram_tensor("gmask", [C, GROUPS], f32, kind="ExternalInput")
    gmt_d = nc.dram_tensor("gmask_t", [GROUPS, C], f32, kind="ExternalInput")
    ones_d = nc.dram_tensor("ones32", [C, D], bf16, kind="ExternalInput")
    out_d = nc.dram_tensor("out", [C, HW], f32, kind="ExternalOutput")

    with tile.TileContext(nc) as tc:
        with (
            tc.tile_pool(name="const", bufs=1) as cpool,
            tc.tile_pool(name="big", bufs=1) as big,
            tc.tile_pool(name="small", bufs=2) as small,
            tc.tile_pool(name="exps", bufs=EPOOL_BUFS) as epool,
            tc.tile_pool(name="epi", bufs=2) as epi,
            tc.tile_pool(name="pseq", bufs=2, space="PSUM") as pseq,
            tc.tile_pool(name="pscore", bufs=2, space="PSUM") as pscore,
            tc.tile_pool(name="pacc", bufs=1, space="PSUM") as pacc,
        ):
            # ---- constants ----
            wq_t = cpool.tile([C, C], f32r, tag="wq")
            wk_t = cpool.tile([C, C], f32r, tag="wk")
            wv_t = cpool.tile([C, C], f32r, tag="wv")
            wo_t = cpool.tile([C, C], f32r, tag="wo")
            b_out = cpool.tile([C, 1], f32, tag="bo")
            gn_kv = cpool.tile([C, 2], f32, tag="gnkv")
            gn_q = cpool.tile([C, 2], f32, tag="gnq")
            gmask = cpool.tile([C, GROUPS], f32, tag="gm")
            gmask_t = cpool.tile([GROUPS, C], f32, tag="gmt")
            eps_t = cpool.tile([C, 1], f32, tag="eps")
            ones32 = cpool.tile([C, D], bf16, tag="ones")
            for t, d in ((wq_t, wq_d), (wk_t, wk_d), (wv_t, wv_d),
                         (wo_t, wo_d), (b_out, bo_d), (gn_kv, gnkv_d),
                         (gn_q, gnq_d), (gmask, gm_d), (gmask_t, gmt_d),
                         (ones32, ones_d)):
                nc.sync.dma_start(out=t[:], in_=d[:])
            nc.vector.memset(eps_t[:], EPS)

            # ---- persistent activations ----
            x_kv = big.tile([C, HW], f32r, tag="x_kv")
            x_q = big.tile([C, HW], f32r, tag="x_q")
            qsb = big.tile([C, HW], bf16, tag="qsb")
            ksb = big.tile([C, HW], bf16, tag="ksb")
            vts = big.tile([C, NKC, C], bf16, tag="vts")  # v^T (hw, [h,d])
            xnb = big.tile([C, HW], f32, tag="xnb")
            nc.sync.dma_start(out=x_kv[:], in_=x_kv_d[:])
            nc.sync.dma_start(out=x_q[:], in_=x_q_d[:])

            def body():
                # ---- GroupNorm (in place) on x_kv and x_q ----
                for x_sb, gn in ((x_kv, gn_kv), (x_q, gn_q)):
                    xs = x_sb[:].rearrange("p (s f) -> p s f", f=512)
                    stats = small.tile([C, 8, 6], f32, tag="gnstats")
                    for s in range(8):
                        nc.vector.bn_stats(out=stats[:, s, :], in_=xs[:, s, :])
                    mv = small.tile([C, 2], f32, tag="gnmv")
                    nc.vector.bn_aggr(out=mv[:], in_=stats[:])
                    # per-channel [mean, E[x^2]]
                    st2 = small.tile([C, 2], f32, tag="gnst2")
                    nc.vector.tensor_copy(st2[:, 0:1], mv[:, 0:1])
                    nc.vector.tensor_tensor(st2[:, 1:2], mv[:, 0:1],
                                            mv[:, 0:1], OP.mult)
                    nc.vector.tensor_tensor(st2[:, 1:2], st2[:, 1:2],
                                            mv[:, 1:2], OP.add)
                    # group combine: (16, 2) = gmask.T @ st2
                    pg = pseq.tile([GROUPS, 2], f32, tag="seq")
                    nc.tensor.matmul(pg[:], lhsT=gmask[:], rhs=st2[:],
                                     start=True, stop=True)
                    gstat = small.tile([GROUPS, 2], f32, tag="gstat")
                    nc.vector.tensor_copy(gstat[:], pg[:])
                    # var_g = E[x^2]_g - mean_g^2 ; rstd = 1/sqrt(var+eps)
                    vtmp = small.tile([GROUPS, 1], f32, tag="gvtmp")
                    nc.vector.tensor_tensor(vtmp[:], gstat[:, 0:1],
                                            gstat[:, 0:1], OP.mult)
                    nc.vector.tensor_tensor(gstat[:, 1:2], gstat[:, 1:2],
                                            vtmp[:], OP.subtract)
                    nc.scalar.activation(out=gstat[:, 1:2], in_=gstat[:, 1:2],
                                         func=AF.Sqrt, bias=eps_t[:GROUPS])
                    nc.vector.reciprocal(out=gstat[:, 1:2], in_=gstat[:, 1:2])
                    # broadcast to channels: (128, 2) = gmask_t.T @ gstat
                    pcb = pseq.tile([C, 2], f32, tag="seq")
                    nc.tensor.matmul(pcb[:], lhsT=gmask_t[:], rhs=gstat[:],
                                     start=True, stop=True)
                    # fold affine: s1 = rstd*w ; s2 = b - mean*s1
                    s1 = small.tile([C, 1], f32, tag="gns1")
                    s2 = small.tile([C, 1], f32, tag="gns2")
                    nc.vector.tensor_tensor(s1[:], pcb[:, 1:2], gn[:, 0:1],
                                            OP.mult)
                    nc.vector.tensor_tensor(s2[:], pcb[:, 0:1], s1[:], OP.mult)
                    nc.vector.tensor_tensor(s2[:], gn[:, 1:2], s2[:],
                                            OP.subtract)
                    nc.vector.tensor_scalar(out=x_sb[:], in0=x_sb[:],
                                            scalar1=s1[:], scalar2=s2[:],
                                            op0=OP.mult, op1=OP.add)

                # xnb = xn_kv + b_out: folds the output bias into the
                # residual so the per-chunk epilogue is a single add.
                nc.vector.tensor_scalar_add(xnb[:], x_kv[:], b_out[:])

                # ---- projections ----
                for j in range(NJ):
                    sl = slice(j * NQ, (j + 1) * NQ)
                    pq = pseq.tile([C, NQ], f32, tag="seq")
                    nc.tensor.matmul(pq[:], lhsT=r(wq_t[:]), rhs=r(x_q[:, sl]),
                                     start=True, stop=True)
                    nc.scalar.copy(out=qsb[:, sl], in_=pq[:])
                    pk = pseq.tile([C, NQ], f32, tag="seq")
                    nc.tensor.matmul(pk[:], lhsT=r(wk_t[:]), rhs=r(x_kv[:, sl]),
                                     start=True, stop=True)
                    nc.vector.tensor_copy(ksb[:, sl], pk[:])
                for j in range(NJ):  # v^T in 4 kc-chunks per PSUM bank
                    pv = pseq.tile([C, 4, C], f32, tag="seq")
                    for t in range(4):
                        kc = j * 4 + t
                        nc.tensor.matmul(
                            pv[:, t, :],
                            lhsT=r(x_kv[:, kc * KC:(kc + 1) * KC]),
                            rhs=r(wv_t[:]), start=True, stop=True)
                    nc.vector.tensor_copy(vts[:, j * 4:(j + 1) * 4, :], pv[:])

                # ---- attention ----
                expctr = 0
                for j in range(NJ):
                    qs = slice(j * NQ, (j + 1) * NQ)
                    po = pacc.tile([C, NQ], f32, tag="po")
                    pd = pacc.tile([C, NQ], f32, tag="pd")
                    for kc in range(NKC):
                        ks = slice(kc * KC, (kc + 1) * KC)
                        first, last = kc == 0, kc == NKC - 1
                        for pair in range(2):
                            ps = pscore.tile([C, 2, NQ], f32, tag="ps")
                            for hh in range(2):
                                h = pair * 2 + hh
                                hs = slice(h * D, (h + 1) * D)
                                nc.tensor.matmul(
                                    ps[:, hh, :],
                                    lhsT=r(ksb[hs, ks]),
                                    rhs=r(qsb[hs, qs]),
                                    start=True, stop=True,
                                    tile_position=(h * D, 0))
                            ex = epool.tile([C, 2, NQ], bf16, tag="ex")
                            if (expctr * ACT_OF_8) % 8 < ACT_OF_8:
                                nc.scalar.activation(out=ex[:], in_=ps[:],
                                                     func=AF.Exp)
                            else:
                                nc.vector._custom_dve(
                                    exp4, out=ex[:], in0=ps[:],
                                    s0=1.0 / 24.0, s1=1.0 / 6.0, imm2=0.5)
                            expctr += 1
                            for hh in range(2):
                                h = pair * 2 + hh
                                hs = slice(h * D, (h + 1) * D)
                                nc.tensor.matmul(
                                    po[hs, :],
                                    lhsT=r(vts[:, kc, hs]),
                                    rhs=r(ex[:, hh, :]),
                                    start=first, stop=last,
                                    skip_group_check=True,
                                    tile_position=(0, h * D))
                                nc.tensor.matmul(
                                    pd[hs, :],
                                    lhsT=r(ones32[:]),
                                    rhs=r(ex[:, hh, :]),
                                    start=first, stop=last,
                                    skip_group_check=True,
                                    tile_position=(0, h * D))
                    # ---- epilogue for this q-chunk ----
                    recip = epi.tile([C, NQ], f32, tag="recip")
                    nc.vector.reciprocal_approx_fast(out=recip[:], in_=pd[:])
                    onorm = epi.tile([C, NQ], f32r, tag="onorm")
                    nc.vector.tensor_tensor(onorm[:], po[:], recip[:], OP.mult)
                    pout = pseq.tile([C, NQ], f32, tag="seq")
                    nc.tensor.matmul(pout[:], lhsT=r(wo_t[:]), rhs=r(onorm[:]),
                                     start=True, stop=True)
                    ot = epi.tile([C, NQ], f32, tag="ot")
                    nc.vector.tensor_tensor(ot[:], pout[:], xnb[:, qs], OP.add)
                    nc.sync.dma_start(out=out_d[:, qs], in_=ot[:])

            if repeat == 1:
                body()
            else:
                with tc.For_i(0, repeat, 1):
                    body()

    nc.compile()
    return nc


import ml_dtypes
_bf16_np = ml_dtypes.bfloat16


def _prep_in_maps(x_A, x_B, gnA_w, gnA_b, gnB_w, gnB_b, W_qkv_A, W_qkv_B,
                  W_out_A, b_out_A, W_out_B, b_out_B):
    """Host-side shard: one in_map per core; constants pre-transposed."""
    f = np.float32
    # bn stats are already means over HW; the group combine averages the 8
    # member channels of each group, so the combine weight is 1/8.
    gmask = np.zeros((C, GROUPS), f)
    gmask_t = np.zeros((GROUPS, C), f)
    for c in range(C):
        gmask[c, c // 8] = 1.0 / 8.0
        gmask_t[c // 8, c] = 1.0
    scale = 1.0 / math.sqrt(C)

    def side_maps(x_self, x_other, Wqkv_self, Wqkv_other, gn_s_w, gn_s_b,
                  gn_o_w, gn_o_b, W_out, b_out):
        qkv_s = np.asarray(Wqkv_self, f).reshape(N_HEAD, 3, D, C)
        qkv_o = np.asarray(Wqkv_other, f).reshape(N_HEAD, 3, D, C)
        wq_t = np.ascontiguousarray(
            (qkv_o[:, 0].reshape(C, C) * scale).T)
        wk_t = np.ascontiguousarray(qkv_s[:, 1].reshape(C, C).T)
        wv_t = np.ascontiguousarray(qkv_s[:, 2].reshape(C, C).T)
        wo_t = np.ascontiguousarray(np.asarray(W_out, f).T)
        common = {
            "wq_t": wq_t, "wk_t": wk_t, "wv_t": wv_t, "wo_t": wo_t,
            "b_out": np.asarray(b_out, f).reshape(C, 1),
            "gn_kv": np.stack([np.asarray(gn_s_w, f),
                               np.asarray(gn_s_b, f)], axis=1),
            "gn_q": np.stack([np.asarray(gn_o_w, f),
                              np.asarray(gn_o_b, f)], axis=1),
            "gmask": gmask, "gmask_t": gmask_t,
            "ones32": np.ones((C, D), _bf16_np),
        }
        return [
            dict(common,
                 x_kv=np.ascontiguousarray(
                     np.asarray(x_self[b], f).reshape(C, HW)),
                 x_q=np.ascontiguousarray(
                     np.asarray(x_other[b], f).reshape(C, HW)))
            for b in range(B)
        ]

    maps = side_maps(x_A, x_B, W_qkv_A, W_qkv_B, gnA_w, gnA_b, gnB_w, gnB_b,
                     W_out_A, b_out_A)
    maps += side_maps(x_B, x_A, W_qkv_B, W_qkv_A, gnB_w, gnB_b, gnA_w, gnA_b,
                      W_out_B, b_out_B)
    return maps


def get_compiled(repeat: int = 1):
    if repeat not in _COMPILED:
        _COMPILED[repeat] = build_nc(repeat)
    return _COMPILED[repeat]


def run_on_cores(in_maps, repeat: int = 1):
    from concourse.bass_utils import run_bass_kernel_spmd
    nc = get_compiled(repeat)
    res = run_bass_kernel_spmd(nc, in_maps, core_ids=list(range(N_CORES)))
    return res.results


def kernel(**inputs):
    in_maps = _prep_in_maps(**{k: np.asarray(v) for k, v in inputs.items()})
    results = run_on_cores(in_maps, repeat=int(os.environ.get("CA_REPEAT", "1")))
    out_A = np.stack([results[b]["out"].reshape(C, 64, 64) for b in range(B)])
    out_B = np.stack([results[B + b]["out"].reshape(C, 64, 64)
                      for b in range(B)])
    return out_A.astype(np.float32), out_B.astype(np.float32)


# revision 14
# speedup vs baseline: 6.6999x; 2.3651x over previous
"""Trainium2 Bass kernel for nn_Cross_attention (dual-stream cross attention).

Shape summary (per stream): x (B=4, C=128, 64, 64) -> GroupNorm(16 groups) ->
QKV 1x1 conv (4 heads, d=32) -> cross attention over HW=4096 positions
(q from the *other* stream) -> out 1x1 conv + bias -> + normalized input.

Sharding: 8 independent (batch, output-side) units -> one per NeuronCore.
Core i < 4 computes out_A[i] (k/v/residual from stream A, q from stream B);
core i >= 4 computes out_B[i-4].

Per-core dataflow (all matmuls float32r, full 128x128 PE rate at N>=256):
  - GroupNorm stats via bn_stats/bn_aggr per channel, group-combined and
    broadcast back with tiny mask matmuls; affine folded into (x*s1 + s2).
  - q, k projected into (128=[head,d], 4096) layout; v projected transposed
    into (4096, 128=[head,d]) layout (lhsT for the PV matmul).
  - Scores computed transposed: S^T(k,q) = k^T q via 4 row-tiled (K=32)
    matmuls, one PE pass for all 4 heads.
  - exp() with no max-subtraction (scores are provably tiny: |s| <~ 0.3);
    split between ScalarE table-exp and a single-pass custom-DVE quartic
    polynomial so both engines stream the 67M-element attention matrix.
  - O = (exp S^T)^T-contracted with v via col-tiled matmuls; softmax
    denominators via a parallel ones-matmul into a second PSUM bank
    (replicated across each head's 32 partitions); normalize with
    reciprocal_approx_fast at the (128, 512) output tile level.
  - out = W_out @ (O/den) + b + xn, DMA'd back per 512-column chunk.
"""

import math
import os
import sys

sys.path.insert(0, "/opt/trn_rl_repo")

import numpy as np

C = 128
N_HEAD = 4
D = 32
GROUPS = 16
EPS = 1e-5
B = 4
HW = 4096
NQ = 512            # q-chunk (free dim of score matmuls / PSUM bank)
NJ = HW // NQ       # 8 q-chunks
KC = 128            # k-chunk (partition dim of exp'd score tiles)
NKC = HW // KC      # 32 k-chunks
N_CORES = 8

# exp engine split: of every 8 (kc, head-pair) chunks, this many go to ScalarE
# (table exp), the rest to the VectorE quartic-poly custom op.
ACT_OF_8 = int(os.environ.get("CA_ACT_OF_8", "5"))
EPOOL_BUFS = int(os.environ.get("CA_EPOOL_BUFS", "6"))

_EXP4 = None
_COMPILED = {}


def _register_exp4():
    """Register a quartic-Taylor exp as a custom DVE op (single uop pass).

    exp(x) ~= 1 + x(1 + x(1/2 + x(1/6 + x/24))); scores satisfy |x| <~ 0.3
    so max rel err ~4e-5 (4e-4 at |x|=0.5) - far below fp32r matmul noise.
    """
    global _EXP4
    if _EXP4 is not None:
        return _EXP4
    from concourse import dve_ops
    from concourse.dve_spec import C0, C1, C2, One, Spec, Src0, lower
    from concourse.dve_uop import DveOpSpec

    for op in dve_ops.OPS:
        if op.name == "EXP4_ANT":
            _EXP4 = op
            return op

    body = ((((Src0 * C0 + C1) * Src0 + C2) * Src0 + One) * Src0 + One)
    spec = Spec(
        body=body,
        reference=lambda in0, in1, s0, s1, imm2: (
            (((in0.astype(np.float32) * s0 + s1) * in0 + imm2) * in0 + 1.0)
            * in0 + 1.0
        ),
    )
    row = dve_ops._CUSTOM_DVE_ROW_BASE + len(dve_ops.OPS)
    assert row < 0x20
    dve_ops._SUB_OPCODE_FOR_NAME["EXP4_ANT"] = row
    shas = {}
    for ver in ("v3", "v4"):
        uops = lower(spec, ver=ver)
        shas[ver] = DveOpSpec(
            name="EXP4_ANT", opcode=row, uops=uops, rd1_en=False
        ).sha(ver)
    op = dve_ops.DveOp("EXP4_ANT", spec, subdim=False, uops_sha=shas)
    dve_ops.OPS.append(op)
    dve_ops.CUSTOM_DVE_SPECS["EXP4_ANT"] = spec
    _EXP4 = op
    return op


def build_nc(repeat: int = 1, act_of_8: int | None = None,
             epool_bufs: int | None = None, no_den: bool = False):
    """Build + compile the SPMD single-core program (same for all 8 cores)."""
    import concourse.bacc as bacc
    import concourse.tile as tile
    from concourse import mybir

    if act_of_8 is None:
        act_of_8 = ACT_OF_8
    if epool_bufs is None:
        epool_bufs = EPOOL_BUFS
    exp4 = _register_exp4()
    f32 = mybir.dt.float32
    f32r = mybir.dt.float32r
    bf16 = mybir.dt.bfloat16
    AF = mybir.ActivationFunctionType
    OP = mybir.AluOpType

    def r(ap):
        return ap

    nc = bacc.Bacc("TRN2", target_bir_lowering=False, debug=False,
                   num_devices=N_CORES)

    x_kv_d = nc.dram_tensor("x_kv", [C, HW], f32r, kind="ExternalInput")
    x_q_d = nc.dram_tensor("x_q", [C, HW], f32r, kind="ExternalInput")
    wq_d = nc.dram_tensor("wq_t", [C, C], f32r, kind="ExternalInput")
    wk_d = nc.dram_tensor("wk_t", [C, C], f32r, kind="ExternalInput")
    wv_d = nc.dram_tensor("wv_t", [C, C], f32r, kind="ExternalInput")
    wo_d = nc.dram_tensor("wo_t", [C, C], f32r, kind="ExternalInput")
    bo_d = nc.dram_tensor("b_out", [C, 1], f32, kind="ExternalInput")
    gnkv_d = nc.dram_tensor("gn_kv", [C, 2], f32, kind="ExternalInput")
    gnq_d = nc.dram_tensor("gn_q", [C, 2], f32, kind="ExternalInput")
    gm_d = nc.dram_tensor("gmask", [C, GROUPS], f32, kind="ExternalInput")
    gmt_d = nc.dram_tensor("gmask_t", [GROUPS, C], f32, kind="ExternalInput")
    ones_d = nc.dram_tensor("ones32", [C, D], bf16, kind="ExternalInput")
    out_d = nc.dram_tensor("out", [C, HW], f32, kind="ExternalOutput")

    with tile.TileContext(nc) as tc:
        with (
            tc.tile_pool(name="const", bufs=1) as cpool,
            tc.tile_pool(name="big", bufs=1) as big,
            tc.tile_pool(name="small", bufs=2) as small,
            tc.tile_pool(name="exps", bufs=epool_bufs) as epool,
            tc.tile_pool(name="epi", bufs=2) as epi,
            tc.tile_pool(name="pseq", bufs=2, space="PSUM") as pseq,
            tc.tile_pool(name="pscore", bufs=2, space="PSUM") as pscore,
            tc.tile_pool(name="pacc", bufs=1, space="PSUM") as pacc,
        ):
            # ---- constants ----
            wq_t = cpool.tile([C, C], f32r, tag="wq")
            wk_t = cpool.tile([C, C], f32r, tag="wk")
            wv_t = cpool.tile([C, C], f32r, tag="wv")
            wo_t = cpool.tile([C, C], f32r, tag="wo")
            b_out = cpool.tile([C, 1], f32, tag="bo")
            gn_kv = cpool.tile([C, 2], f32, tag="gnkv")
            gn_q = cpool.tile([C, 2], f32, tag="gnq")
            gmask = cpool.tile([C, GROUPS], f32, tag="gm")
            gmask_t = cpool.tile([GROUPS, C], f32, tag="gmt")
            eps_t = cpool.tile([C, 1], f32, tag="eps")
            ones32 = cpool.tile([C, D], bf16, tag="ones")
            for t, d in ((wq_t, wq_d), (wk_t, wk_d), (wv_t, wv_d),
                         (wo_t, wo_d), (b_out, bo_d), (gn_kv, gnkv_d),
                         (gn_q, gnq_d), (gmask, gm_d), (gmask_t, gmt_d),
                         (ones32, ones_d)):
                nc.sync.dma_start(out=t[:], in_=d[:])
            nc.vector.memset(eps_t[:], EPS)

            # ---- persistent activations ----
            x_kv = big.tile([C, HW], f32r, tag="x_kv")
            x_q = big.tile([C, HW], f32r, tag="x_q")
            qsb = big.tile([C, HW], bf16, tag="qsb")
            ksb = big.tile([C, HW], bf16, tag="ksb")
            vts = big.tile([C, NKC, C], bf16, tag="vts")  # v^T (hw, [h,d])
            xnb = big.tile([C, HW], f32, tag="xnb")
            nc.sync.dma_start(out=x_kv[:], in_=x_kv_d[:])
            nc.sync.dma_start(out=x_q[:], in_=x_q_d[:])

            def body():
                # ---- GroupNorm (in place) on x_kv and x_q ----
                for x_sb, gn in ((x_kv, gn_kv), (x_q, gn_q)):
                    xs = x_sb[:].rearrange("p (s f) -> p s f", f=512)
                    stats = small.tile([C, 8, 6], f32, tag="gnstats")
                    for s in range(8):
                        nc.vector.bn_stats(out=stats[:, s, :], in_=xs[:, s, :])
                    mv = small.tile([C, 2], f32, tag="gnmv")
                    nc.vector.bn_aggr(out=mv[:], in_=stats[:])
                    # per-channel [mean, E[x^2]]
                    st2 = small.tile([C, 2], f32, tag="gnst2")
                    nc.vector.tensor_copy(st2[:, 0:1], mv[:, 0:1])
                    nc.vector.tensor_tensor(st2[:, 1:2], mv[:, 0:1],
                                            mv[:, 0:1], OP.mult)
                    nc.vector.tensor_tensor(st2[:, 1:2], st2[:, 1:2],
                                            mv[:, 1:2], OP.add)
                    # group combine: (16, 2) = gmask.T @ st2
                    pg = pseq.tile([GROUPS, 2], f32, tag="seq")
                    nc.tensor.matmul(pg[:], lhsT=gmask[:], rhs=st2[:],
                                     start=True, stop=True)
                    gstat = small.tile([GROUPS, 2], f32, tag="gstat")
                    nc.vector.tensor_copy(gstat[:], pg[:])
                    # var_g = E[x^2]_g - mean_g^2 ; rstd = 1/sqrt(var+eps)
                    vtmp = small.tile([GROUPS, 1], f32, tag="gvtmp")
                    nc.vector.tensor_tensor(vtmp[:], gstat[:, 0:1],
                                            gstat[:, 0:1], OP.mult)
                    nc.vector.tensor_tensor(gstat[:, 1:2], gstat[:, 1:2],
                                            vtmp[:], OP.subtract)
                    nc.scalar.activation(out=gstat[:, 1:2], in_=gstat[:, 1:2],
                                         func=AF.Sqrt, bias=eps_t[:GROUPS])
                    nc.vector.reciprocal(out=gstat[:, 1:2], in_=gstat[:, 1:2])
                    # broadcast to channels: (128, 2) = gmask_t.T @ gstat
                    pcb = pseq.tile([C, 2], f32, tag="seq")
                    nc.tensor.matmul(pcb[:], lhsT=gmask_t[:], rhs=gstat[:],
                                     start=True, stop=True)
                    # fold affine: s1 = rstd*w ; s2 = b - mean*s1
                    s1 = small.tile([C, 1], f32, tag="gns1")
                    s2 = small.tile([C, 1], f32, tag="gns2")
                    nc.vector.tensor_tensor(s1[:], pcb[:, 1:2], gn[:, 0:1],
                                            OP.mult)
                    nc.vector.tensor_tensor(s2[:], pcb[:, 0:1], s1[:], OP.mult)
                    nc.vector.tensor_tensor(s2[:], gn[:, 1:2], s2[:],
                                            OP.subtract)
                    nc.vector.tensor_scalar(out=x_sb[:], in0=x_sb[:],
                                            scalar1=s1[:], scalar2=s2[:],
                                            op0=OP.mult, op1=OP.add)

                # xnb = xn_kv + b_out: folds the output bias into the
                # residual so the per-chunk epilogue is a single add.
                nc.vector.tensor_scalar_add(xnb[:], x_kv[:], b_out[:])

                # ---- projections ----
                for j in range(NJ):
                    sl = slice(j * NQ, (j + 1) * NQ)
                    pq = pseq.tile([C, NQ], f32, tag="seq")
                    nc.tensor.matmul(pq[:], lhsT=r(wq_t[:]), rhs=r(x_q[:, sl]),
                                     start=True, stop=True)
                    nc.scalar.copy(out=qsb[:, sl], in_=pq[:])
                    pk = pseq.tile([C, NQ], f32, tag="seq")
                    nc.tensor.matmul(pk[:], lhsT=r(wk_t[:]), rhs=r(x_kv[:, sl]),
                                     start=True, stop=True)
                    nc.vector.tensor_copy(ksb[:, sl], pk[:])
                for j in range(NJ):  # v^T in 4 kc-chunks per PSUM bank
                    pv = pseq.tile([C, 4, C], f32, tag="seq")
                    for t in range(4):
                        kc = j * 4 + t
                        nc.tensor.matmul(
                            pv[:, t, :],
                            lhsT=r(x_kv[:, kc * KC:(kc + 1) * KC]),
                            rhs=r(wv_t[:]), start=True, stop=True)
                    nc.vector.tensor_copy(vts[:, j * 4:(j + 1) * 4, :], pv[:])

                # ---- attention ----
                expctr = 0
                for j in range(NJ):
                    qs = slice(j * NQ, (j + 1) * NQ)
                    po = pacc.tile([C, NQ], f32, tag="po")
                    pd = None if no_den else pacc.tile([C, NQ], f32, tag="pd")

                    def emit_pv(kc, exs, po=po, pd=pd):
                        # Second-stage matmuls for k-chunk kc, one chunk
                        # behind the score matmuls so PE streams these while
                        # ACT/DVE exponentiate the next chunk. All four po
                        # matmuls go first (4 distinct col strips, concurrent)
                        # then the four pd ones - interleaving po/pd per head
                        # would convoy on the same strip back-to-back.
                        first, last = kc == 0, kc == NKC - 1
                        for h in range(N_HEAD):
                            hs = slice(h * D, (h + 1) * D)
                            nc.tensor.matmul(
                                po[hs, :],
                                lhsT=r(vts[:, kc, hs]),
                                rhs=r(exs[h // 2][:, h % 2, :]),
                                start=first, stop=last,
                                skip_group_check=True,
                                tile_position=(0, h * D))
                        if no_den:
                            return
                        for h in range(N_HEAD):
                            hs = slice(h * D, (h + 1) * D)
                            nc.tensor.matmul(
                                pd[hs, :],
                                lhsT=r(ones32[:]),
                                rhs=r(exs[h // 2][:, h % 2, :]),
                                start=first, stop=last,
                                skip_group_check=True,
                                tile_position=(0, h * D))

                    prev = None
                    for kc in range(NKC):
                        ks = slice(kc * KC, (kc + 1) * KC)
                        cur = []
                        for pair in range(2):
                            ps = pscore.tile([C, 2, NQ], f32, tag="ps")
                            for hh in range(2):
                                h = pair * 2 + hh
                                hs = slice(h * D, (h + 1) * D)
                                nc.tensor.matmul(
                                    ps[:, hh, :],
                                    lhsT=r(ksb[hs, ks]),
                                    rhs=r(qsb[hs, qs]),
                                    start=True, stop=True,
                                    tile_position=(h * D, 0))
                            ex = epool.tile([C, 2, NQ], bf16, tag="ex")
                            if (expctr * act_of_8) % 8 < act_of_8:
                                nc.scalar.activation(out=ex[:], in_=ps[:],
                                                     func=AF.Exp)
                            else:
                                nc.vector._custom_dve(
                                    exp4, out=ex[:], in0=ps[:],
                                    s0=1.0 / 24.0, s1=1.0 / 6.0, imm2=0.5)
                            expctr += 1
                            cur.append(ex)
                        if prev is not None:
                            emit_pv(kc - 1, prev)
                        prev = cur
                    emit_pv(NKC - 1, prev)
                    # ---- epilogue for this q-chunk ----
                    recip = epi.tile([C, NQ], f32, tag="recip")
                    if no_den:
                        nc.vector.memset(recip[:], 1.0 / HW)
                    else:
                        nc.vector.reciprocal_approx_fast(out=recip[:], in_=pd[:])
                    onorm = epi.tile([C, NQ], f32r, tag="onorm")
                    nc.vector.tensor_tensor(onorm[:], po[:], recip[:], OP.mult)
                    pout = pseq.tile([C, NQ], f32, tag="seq")
                    nc.tensor.matmul(pout[:], lhsT=r(wo_t[:]), rhs=r(onorm[:]),
                                     start=True, stop=True)
                    ot = epi.tile([C, NQ], f32, tag="ot")
                    nc.vector.tensor_tensor(ot[:], pout[:], xnb[:, qs], OP.add)
                    nc.sync.dma_start(out=out_d[:, qs], in_=ot[:])

            if repeat == 1:
                body()
            else:
                with tc.For_i(0, repeat, 1):
                    body()

    nc.compile()
    return nc


import ml_dtypes
_bf16_np = ml_dtypes.bfloat16


def _prep_in_maps(x_A, x_B, gnA_w, gnA_b, gnB_w, gnB_b, W_qkv_A, W_qkv_B,
                  W_out_A, b_out_A, W_out_B, b_out_B):
    """Host-side shard: one in_map per core; constants pre-transposed."""
    f = np.float32
    # bn stats are already means over HW; the group combine averages the 8
    # member channels of each group, so the combine weight is 1/8.
    gmask = np.zeros((C, GROUPS), f)
    gmask_t = np.zeros((GROUPS, C), f)
    for c in range(C):
        gmask[c, c // 8] = 1.0 / 8.0
        gmask_t[c // 8, c] = 1.0
    scale = 1.0 / math.sqrt(C)

    def side_maps(x_self, x_other, Wqkv_self, Wqkv_other, gn_s_w, gn_s_b,
                  gn_o_w, gn_o_b, W_out, b_out):
        qkv_s = np.asarray(Wqkv_self, f).reshape(N_HEAD, 3, D, C)
        qkv_o = np.asarray(Wqkv_other, f).reshape(N_HEAD, 3, D, C)
        wq_t = np.ascontiguousarray(
            (qkv_o[:, 0].reshape(C, C) * scale).T)
        wk_t = np.ascontiguousarray(qkv_s[:, 1].reshape(C, C).T)
        wv_t = np.ascontiguousarray(qkv_s[:, 2].reshape(C, C).T)
        wo_t = np.ascontiguousarray(np.asarray(W_out, f).T)
        common = {
            "wq_t": wq_t, "wk_t": wk_t, "wv_t": wv_t, "wo_t": wo_t,
            "b_out": np.asarray(b_out, f).reshape(C, 1),
            "gn_kv": np.stack([np.asarray(gn_s_w, f),
                               np.asarray(gn_s_b, f)], axis=1),
            "gn_q": np.stack([np.asarray(gn_o_w, f),
                              np.asarray(gn_o_b, f)], axis=1),
            "gmask": gmask, "gmask_t": gmask_t,
            "ones32": np.ones((C, D), _bf16_np),
        }
        return [
            dict(common,
                 x_kv=np.ascontiguousarray(
                     np.asarray(x_self[b], f).reshape(C, HW)),
                 x_q=np.ascontiguousarray(
                     np.asarray(x_other[b], f).reshape(C, HW)))
            for b in range(B)
        ]

    maps = side_maps(x_A, x_B, W_qkv_A, W_qkv_B, gnA_w, gnA_b, gnB_w, gnB_b,
                     W_out_A, b_out_A)
    maps += side_maps(x_B, x_A, W_qkv_B, W_qkv_A, gnB_w, gnB_b, gnA_w, gnA_b,
                      W_out_B, b_out_B)
    return maps


def get_compiled(repeat: int = 1):
    if repeat not in _COMPILED:
        _COMPILED[repeat] = build_nc(repeat)
    return _COMPILED[repeat]


def run_on_cores(in_maps, repeat: int = 1):
    from concourse.bass_utils import run_bass_kernel_spmd
    nc = get_compiled(repeat)
    res = run_bass_kernel_spmd(nc, in_maps, core_ids=list(range(N_CORES)))
    return res.results


def kernel(**inputs):
    in_maps = _prep_in_maps(**{k: np.asarray(v) for k, v in inputs.items()})
    results = run_on_cores(in_maps, repeat=int(os.environ.get("CA_REPEAT", "1")))
    out_A = np.stack([results[b]["out"].reshape(C, 64, 64) for b in range(B)])
    out_B = np.stack([results[B + b]["out"].reshape(C, 64, 64)
                      for b in range(B)])
    return out_A.astype(np.float32), out_B.astype(np.float32)
